# revision 31
# baseline (speedup 1.0000x reference)
"""Trainium2 Bass kernel for nn_DIAGCN (RGCN + GraphConv + classifier over
block-diagonal dialog graphs), SPMD over 8 NeuronCores.

Strategy
--------
The dialog graph is a causal 5-tap window (edges i -> i+o, o = 0..4, within
each 100-utterance dialog), and relation_type(i,j) = spk[i]*spk[j] with spk
derived from self-edges.  Every per-node linear map commutes with both the
window sum W(.) (row-mixing) and per-node diagonal scalings, so the whole
network folds into 7-wide channels:

    out = W(g0) + f0
    g0  = RA + ic0.*W(A0) - ic0s.*W(spk.*A0) + ic1s.*W(spk.*A1) + cA*nv.*mask
    f0  = FSC + ic0.*W(B0) - ic0s.*W(spk.*B0) + ic1s.*W(spk.*B1) + cBc.*mask
    A{0,1} = x@(w_rel{0,1}@wA), B likewise with wB; RA = x@(w_root@wA),
    FSC = x@(w_root@wB + w_skip@w_clf); wA = w_gc_rel@w_clf, wB = w_gc_root@w_clf

Device work per column tile: 8 k-block matmuls [1024 -> 78] over x (the
memory-bound stream; k-PAIRED across the two tiles of a pair so each
LDWEIGHTS serves two matmuls and hides in the PE reorder window), one DVE
multiply ps[0:42] * [spk|ones] -> tZ[0:42], one ACT copy ps[64:78] ->
tU[64:78] (RA|FSC; ps rows 42:64 are zero columns of Wbig so every engine
slice starts at a legal partition base), the 5-tap window shift-tree per
pair, a coefficient multiply window*coef -> tU[0:42], and ONE [80 -> 39]
S-matmul per tile over tU (V rows + RA/FSC + nvm/mask const rows -> g0, f0),
deferred one pair so the PE queue never waits on the DVE chain.  ACT copies
(g0, f0) into 4-group packed planes for the final packed 5-tap window (win2);
the packed [128, 1664] output plane ships in chunked DMAs that overlap the
tail, and the host unpacks.  Measured: ~71.5-73.6 us on HW (baseline 75.7),
rel err 4.7e-3 (gate 2e-2).

Layout: nodes sharded by dialog (no cross-core edges), 64 padded dialogs per
core; each dialog = 4 zero gap columns + 100 data columns so window sums
never leak across dialogs.  x ships transposed+tiled bf16, one 1 MB DMA per
column tile on the sync HWDGE ring with a shallow buffer-gated queue
(concurrent DMAs share HBM round-robin, so a deep queue delays the first
tile); constants stream in parallel on the scalar ring, weights first.
"""
import numpy as np
import ml_dtypes

BF16 = ml_dtypes.bfloat16
FP8 = ml_dtypes.float8_e4m3

# ---------------------------------------------------------------- constants
B, L, FUT = 500, 100, 4
N = B * L
IN, HID, NCLS = 1024, 512, 7
NCORES = 8
GAP = 4
DLG = L + GAP            # 104 columns per dialog
DPC = 64                 # padded dialogs per core
COLS = DPC * DLG         # 6656 columns per core
NT = 13                  # column tiles
NTC = COLS // NT         # 512
KB = IN // 128           # 8 contraction blocks
M = 78                   # Wbig columns / psum rows (42:64 zero)
MS = 80                  # S-matmul contraction rows
M2 = 39                  # S-matmul output columns (g0 at 0:7, f0 at 32:39)
WIN = 42                 # windowed rows (0:28 spk-scaled, 28:42 plain)
GRP = COLS // 4          # packed-output group width

# ps rows: 0:7 A0(->A0S) 7:14 A1(->A1S) 14:21 B0(->B0S) 21:28 B1(->B1S)
#          28:35 A0 plain, 35:42 B0 plain, 42:64 ZERO, 64:71 RA, 71:78 FSC
# tZ rows (win input): 0:42 = ps[0:42] * [spk*28|ones*14]
# tU rows (S-mm rhs):  0:42 V = window(tZ)*coef, 42:64 zeros (memset once),
#                      64:78 [RA|FSC], 78:80 [nvm|mask] consts

D_COUNTS = [63, 63, 63, 63, 62, 62, 62, 62]
D_STARTS = np.concatenate([[0], np.cumsum(D_COUNTS)])[:-1]


def _data_cols():
    d = np.arange(DPC)[:, None]
    u = np.arange(L)[None, :]
    return d * DLG + GAP + u  # [DPC, L]


# ---------------------------------------------------------------- host prep
def _check_graph(edges, relation_type):
    i = np.arange(L)[:, None]
    off = np.arange(FUT + 1)[None, :]
    tl = i + off
    valid = tl < L
    sl = np.broadcast_to(i, tl.shape)[valid]
    tl = tl[valid]
    base = (np.arange(B) * L)[:, None]
    src = (base + sl[None, :]).reshape(-1)
    tgt = (base + tl[None, :]).reshape(-1)
    if edges.shape != (2, src.size) or not (
        np.array_equal(edges[0], src) and np.array_equal(edges[1], tgt)
    ):
        raise ValueError("edge structure does not match the DIAGCN pattern")
    sel = edges[0] == edges[1]
    spk = np.zeros(N, dtype=np.float64)
    spk[edges[0][sel]] = relation_type[sel]
    return spk


def _host_prep(x, edges, relation_type, w_rel, w_root, b_rgcn,
               w_gc_rel, w_gc_root, b_gc, w_skip, b_skip, w_clf, b_clf):
    x = np.asarray(x, dtype=np.float32)
    edges = np.asarray(edges)
    relation_type = np.asarray(relation_type)
    spk = _check_graph(edges, relation_type)

    tgt = edges[1]
    c1 = np.bincount(tgt[relation_type == 1], minlength=N).astype(np.float64)
    c0 = np.bincount(tgt[relation_type == 0], minlength=N).astype(np.float64)
    ic0 = 1.0 / np.maximum(c0, 1.0)
    ic1 = 1.0 / np.maximum(c1, 1.0)
    ic0s = ic0 * spk
    ic1s = ic1 * spk

    f8 = lambda a: np.asarray(a, dtype=np.float64)
    w_rel, w_root, w_gc_rel, w_gc_root, w_skip, w_clf = map(
        f8, (w_rel, w_root, w_gc_rel, w_gc_root, w_skip, w_clf))
    b_rgcn, b_gc, b_skip, b_clf = map(f8, (b_rgcn, b_gc, b_skip, b_clf))

    wA = w_gc_rel @ w_clf
    wB = w_gc_root @ w_clf
    Wbig = np.zeros((IN, M), dtype=np.float64)
    Wbig[:, 0:7] = w_rel[0] @ wA
    Wbig[:, 7:14] = w_rel[1] @ wA
    Wbig[:, 14:21] = w_rel[0] @ wB
    Wbig[:, 21:28] = w_rel[1] @ wB
    Wbig[:, 28:35] = w_rel[0] @ wA
    Wbig[:, 35:42] = w_rel[0] @ wB
    # 42:64 zero
    Wbig[:, 64:71] = w_root @ wA
    Wbig[:, 71:78] = w_root @ wB + w_skip @ w_clf
    # [128 partitions, KB, M]: partition p holds weight rows {k*128+p}
    Wbig = np.ascontiguousarray(
        Wbig.reshape(KB, 128, M).swapaxes(0, 1)).astype(BF16)

    cA = b_rgcn @ wA
    cBc = b_rgcn @ wB + (b_gc + b_skip) @ w_clf + b_clf
    # S-matmul stationary [MS=80, M2=39] over tU
    Sx = np.zeros((MS, M2), dtype=np.float32)
    for i in range(7):
        Sx[0 + i, i] = 1.0           # -ic0s.*W(A0S)
        Sx[7 + i, i] = 1.0           # +ic1s.*W(A1S)
        Sx[14 + i, 32 + i] = 1.0     # -ic0s.*W(B0S)
        Sx[21 + i, 32 + i] = 1.0     # +ic1s.*W(B1S)
        Sx[28 + i, i] = 1.0          # ic0.*W(A0)
        Sx[35 + i, 32 + i] = 1.0     # ic0.*W(B0)
        Sx[64 + i, i] = 1.0          # RA
        Sx[71 + i, 32 + i] = 1.0     # FSC
    Sx[78, 0:7] = cA                 # nvm row
    Sx[79, 32:39] = cBc              # mask row
    Sx = Sx.astype(BF16)

    dc = _data_cols()
    mask_col = np.zeros(COLS, dtype=np.float64)
    mask_col[dc.reshape(-1)] = 1.0
    nvm = np.convolve(mask_col, np.ones(FUT + 1))[:COLS] * mask_col
    zc = np.zeros((2, COLS), dtype=np.float32)   # -> tU rows 78:80
    zc[0] = nvm
    zc[1] = mask_col
    zc = zc.astype(BF16)

    in_maps = []
    unshard_info = []
    for c in range(NCORES):
        nd = D_COUNTS[c]
        g0 = D_STARTS[c]
        cols_real = dc[:nd].reshape(-1)
        nodes_real = g0 * L + np.arange(nd * L)

        xt = np.zeros((IN, COLS), dtype=np.float32)
        xt[:, cols_real] = x[nodes_real].T
        # swizzle: [NT][128 partitions][KB][NTC] so each column tile is one
        # DMA with 8 KiB contiguous per partition
        xts = np.ascontiguousarray(
            xt.reshape(KB, 128, NT, NTC).transpose(2, 1, 0, 3)).astype(BF16)

        def vec_to_cols(v):
            out = np.zeros(COLS, dtype=np.float32)
            out[cols_real] = v[nodes_real]
            return out

        spk_c = vec_to_cols(spk)
        ic0_c = vec_to_cols(ic0)
        ic0s_c = vec_to_cols(ic0s)
        ic1s_c = vec_to_cols(ic1s)

        spk32 = np.empty((32, COLS), dtype=np.float32)
        spk32[0:28] = spk_c
        spk32[28:32] = 1.0  # rows 28:32 of the spk|ones plane (rest memset)
        coefrep = np.zeros((WIN, COLS), dtype=np.float32)
        coefrep[0:7] = -ic0s_c
        coefrep[7:14] = ic1s_c
        coefrep[14:21] = -ic0s_c
        coefrep[21:28] = ic1s_c
        coefrep[28:35] = ic0_c
        coefrep[35:42] = ic0_c

        in_maps.append(dict(
            xt=xts, wbig=Wbig, sx=Sx, zc=zc,
            spk28=spk32.astype(FP8),
            coefrep=coefrep.astype(BF16),
        ))
        unshard_info.append((nodes_real, cols_real))
    return in_maps, unshard_info


# ---------------------------------------------------------------- bass kernel
_COMPILED = None


def _build():
    import concourse.bass as bass
    from concourse import bacc
    import concourse.mybir as mybir
    from concourse.tile import TileContext

    f32 = mybir.dt.float32
    bf16 = mybir.dt.bfloat16
    f8 = mybir.dt.float8e4
    ADD = mybir.AluOpType.add
    MUL = mybir.AluOpType.mult

    nc = bacc.Bacc("TRN2", target_bir_lowering=False, debug=False,
                   num_devices=NCORES)
    xt_d = nc.dram_tensor("xt", [NT, 128, KB, NTC], bf16, kind="ExternalInput")
    wbig_d = nc.dram_tensor("wbig", [128, KB, M], bf16, kind="ExternalInput")
    sx_d = nc.dram_tensor("sx", [MS, M2], bf16, kind="ExternalInput")
    zc_d = nc.dram_tensor("zc", [2, COLS], bf16, kind="ExternalInput")
    spk_d = nc.dram_tensor("spk28", [32, COLS], f8, kind="ExternalInput")
    coef_d = nc.dram_tensor("coefrep", [WIN, COLS], bf16, kind="ExternalInput")
    y_d = nc.dram_tensor("y", [128, GRP], bf16, kind="ExternalOutput")

    with TileContext(nc) as tc:
        with (
            tc.tile_pool(name="const", bufs=1) as cpool,
            tc.tile_pool(name="xin", bufs=6) as xpool,
            tc.tile_pool(name="wrk", bufs=3) as wpool,
            tc.tile_pool(name="g2", bufs=1) as gpool,
            tc.tile_pool(name="psum", bufs=6, space="PSUM") as ppool,
            tc.tile_pool(name="psum2", bufs=2, space="PSUM") as p2pool,
        ):
            # x tile 0 halves + tile 1 go FIRST on the sync ring so the PE
            # pipeline starts as early as possible
            xts = {}
            xts[0] = xpool.tile([128, KB, NTC], bf16, tag="xt", name="xt_t")
            nc.sync.dma_start(xts[0][:, 0:KB // 2, :], xt_d[0, :, 0:KB // 2, :])
            nc.sync.dma_start(xts[0][:, KB // 2:, :], xt_d[0, :, KB // 2:, :])
            for t in (1, 2, 3):
                xts[t] = xpool.tile([128, KB, NTC], bf16, tag="xt", name="xt_t")
                nc.sync.dma_start(xts[t][:, 0:KB // 2, :], xt_d[t, :, 0:KB // 2, :])
                nc.sync.dma_start(xts[t][:, KB // 2:, :], xt_d[t, :, KB // 2:, :])
            # weights lead the scalar ring (they gate the first matmul)
            wsb = cpool.tile([128, KB, M], bf16)
            nc.scalar.dma_start(wsb[:, 0:1, :], wbig_d[:, 0:1, :])
            nc.scalar.dma_start(wsb[:, 1:, :], wbig_d[:, 1:, :])
            sxsb = cpool.tile([MS, M2], bf16)
            nc.scalar.dma_start(sxsb[:], sx_d[:])

            # persistent [*, COLS] planes (SBUF free-dim bytes are charged per
            # partition regardless of row count).  Engine-op APs must start at
            # partition 0/32/64/96 and a non-zero base spans at most 32
            # partitions.
            tZ = cpool.tile([WIN, COLS], bf16)   # window input rows
            tSP = cpool.tile([WIN, COLS], f8)    # spk|ones plane (exact in fp8)
            tT1 = cpool.tile([WIN, COLS], bf16)  # window stage 1 (persists)
            tU = cpool.tile([MS, COLS], bf16)    # S-matmul rhs plane
            tCF = cpool.tile([WIN, COLS], bf16)  # coefficients
            tGP = cpool.tile([128, GRP], bf16)   # packed g0: grp g rows 32g..32g+6
            tOP = cpool.tile([128, GRP], bf16)   # packed out, same layout

            nc.scalar.dma_start(tSP[0:32], spk_d[:])
            nc.scalar.dma_start(tCF[:], coef_d[:])
            nc.scalar.dma_start(tU[78:80], zc_d[:])
            # ones rows of the spk|ones plane + zero rows of tU, off the DVE
            # queue (gpsimd is otherwise idle; both precede first consumers)
            nc.gpsimd.memset(tSP[32:WIN], 1.0)
            nc.gpsimd.memset(tU[32:64], 0.0)     # 32:42 overwritten by V later
            # dummy copy so the one-time ACT table load happens at startup
            nc.scalar.copy(tT1[0:1, 0:8], sxsb[0:1, 0:8])

            def finalize_pair(tlo, thi):
                # one S-matmul per tile over tU into a pair-wide 2-bank ps2,
                # then pair-wide PSUM->packed copies; called one pair late so
                # the PE never waits on the DVE chain
                for t in range(tlo, thi):
                    c0, c1 = t * NTC, (t + 1) * NTC
                    ps2 = p2pool.tile([M2, NTC], f32, tag="ps2", name="ps2")
                    nc.tensor.matmul(ps2[:], sxsb[:], tU[:, c0:c1],
                                     start=True, stop=True)
                    for (glo, ghi) in [(c0, min(c1, (c0 // GRP + 1) * GRP)),
                                       ((c0 // GRP + 1) * GRP, c1)]:
                        if glo >= ghi:
                            continue
                        g = glo // GRP
                        nc.scalar.copy(
                            tGP[32 * g:32 * g + NCLS, glo - g * GRP:ghi - g * GRP],
                            ps2[0:NCLS, glo - c0:ghi - c0])
                        nc.scalar.copy(
                            tOP[32 * g:32 * g + NCLS, glo - g * GRP:ghi - g * GRP],
                            ps2[32:32 + NCLS, glo - c0:ghi - c0])

            # win2: 5-tap window of packed g0, all 4 groups per op (rows
            # 32g..32g+6).  Chunked so most of it overlaps the main loop.
            NR = 96 + NCLS
            gs1 = gpool.tile([NR, GRP], bf16, tag="gs1")
            gp = tGP[0:NR]

            def win2_chunk(b0, b1, eng=None):
                eng = eng or nc.vector
                gs2 = gpool.tile([NR, b1 - b0], bf16, tag="gs2")
                gwt = gpool.tile([NR, b1 - b0], bf16, tag="gwt")
                if b0 == 0:
                    eng.tensor_copy(gs1[:, 0:1], gp[:, 0:1])
                    eng.tensor_tensor(gs1[:, 1:b1], gp[:, 1:b1], gp[:, 0:b1 - 1], ADD)
                    eng.tensor_copy(gs2[:, 0:2], gs1[:, 0:2])
                    eng.tensor_tensor(gs2[:, 2:], gs1[:, 2:b1], gs1[:, 0:b1 - 2], ADD)
                    eng.tensor_copy(gwt[:, 0:4], gs2[:, 0:4])
                    eng.tensor_tensor(gwt[:, 4:], gs2[:, 4:], gp[:, 0:b1 - 4], ADD)
                else:
                    eng.tensor_tensor(gs1[:, b0:b1], gp[:, b0:b1], gp[:, b0 - 1:b1 - 1], ADD)
                    eng.tensor_tensor(gs2[:], gs1[:, b0:b1], gs1[:, b0 - 2:b1 - 2], ADD)
                    eng.tensor_tensor(gwt[:], gs2[:], gp[:, b0 - 4:b1 - 4], ADD)
                eng.tensor_tensor(tOP[0:NR, b0:b1], tOP[0:NR, b0:b1], gwt[:], ADD)

            B1 = 12 * NTC - 3 * GRP  # group-3 columns complete after tile 11

            pend = None
            pairs = [(2 * p, min(2 * p + 2, NT)) for p in range((NT + 1) // 2)]
            for pi, (tlo, thi) in enumerate(pairs):
                for t in range(tlo, thi):
                    if t not in xts:
                        xts[t] = xpool.tile([128, KB, NTC], bf16, tag="xt", name="xt_t")
                        nc.sync.dma_start(xts[t][:], xt_d[t])
                pss = {}
                if pi <= 1:
                    # tile-sequential while the DMA ramps
                    for t in range(tlo, thi):
                        xt_t = xts.pop(t)
                        pss[t] = ppool.tile([M, NTC], f32, tag="ps", name="ps")
                        for k in range(KB):
                            nc.tensor.matmul(
                                pss[t][:], wsb[:, k, :], xt_t[:, k, :],
                                start=(k == 0), stop=(k == KB - 1))
                else:
                    # k-paired: each LDWEIGHTS serves both tiles of the pair
                    xt_p = {t: xts.pop(t) for t in range(tlo, thi)}
                    for t in range(tlo, thi):
                        pss[t] = ppool.tile([M, NTC], f32, tag="ps", name="ps")
                    for k in range(KB):
                        for t in range(tlo, thi):
                            nc.tensor.matmul(
                                pss[t][:], wsb[:, k, :], xt_p[t][:, k, :],
                                start=(k == 0), stop=(k == KB - 1))

                for t in range(tlo, thi):
                    c0, c1 = t * NTC, (t + 1) * NTC
                    # rows 0:28 spk-scaled, 28:42 ones
                    nc.vector.tensor_tensor(tZ[:, c0:c1], pss[t][0:WIN], tSP[:, c0:c1], MUL)
                    # RA|FSC pass-through (ps rows 42:64 are exact zeros);
                    # this is also what frees the main ps bank
                    nc.scalar.copy(tU[64:78, c0:c1], pss[t][64:78])

                # the previous pair's S-matmuls are ready (its V-multiply ran
                # in its own iteration)
                if pend is not None:
                    finalize_pair(*pend)
                    pend = None

                # 5-tap causal window over the pair's columns as a shift tree:
                #   t1 = z + sh1(z); t2 = t1 + sh2(t1); wt = t2 + sh4(z)
                C0, C1 = tlo * NTC, thi * NTC
                W = C1 - C0
                Z = tZ
                T1 = tT1
                T2 = wpool.tile([WIN, W], bf16, tag="T2")
                WT = wpool.tile([WIN, W], bf16, tag="WT")
                if tlo == 0:
                    nc.vector.tensor_copy(T1[:, 0:1], Z[:, 0:1])
                    nc.vector.tensor_tensor(T1[:, 1:C1], Z[:, 1:C1], Z[:, 0:C1 - 1], ADD)
                    nc.vector.tensor_copy(T2[:, 0:2], T1[:, 0:2])
                    nc.vector.tensor_tensor(T2[:, 2:], T1[:, 2:C1], T1[:, 0:C1 - 2], ADD)
                    nc.vector.tensor_copy(WT[:, 0:4], T2[:, 0:4])
                    nc.vector.tensor_tensor(WT[:, 4:], T2[:, 4:], Z[:, 0:C1 - 4], ADD)
                else:
                    nc.vector.tensor_tensor(T1[:, C0:C1], Z[:, C0:C1], Z[:, C0 - 1:C1 - 1], ADD)
                    nc.vector.tensor_tensor(T2[:], T1[:, C0:C1], T1[:, C0 - 2:C1 - 2], ADD)
                    nc.vector.tensor_tensor(WT[:], T2[:], Z[:, C0 - 4:C1 - 4], ADD)
                nc.vector.tensor_tensor(tU[0:WIN, C0:C1], WT[:], tCF[:, C0:C1], MUL)

                pend = (tlo, thi)

            # tiles 0..11 are finalized: the bulk of win2 + the first output
            # chunk overlap tile 12's S-matmul and copies
            win2_chunk(0, B1)
            nc.scalar.dma_start(y_d[:, 0:B1], tOP[:, 0:B1])
            finalize_pair(*pend)
            win2_chunk(B1, GRP)
            MID = (B1 + GRP) // 2
            nc.sync.dma_start(y_d[:, B1:MID], tOP[:, B1:MID])
            nc.scalar.dma_start(y_d[:, MID:GRP], tOP[:, MID:GRP])
    nc.compile()
    return nc


def _get_compiled():
    global _COMPILED
    if _COMPILED is None:
        _COMPILED = _build()
    return _COMPILED


def _run(in_maps, trace=False):
    from concourse.bass_utils import run_bass_kernel_spmd
    nc = _get_compiled()
    return run_bass_kernel_spmd(nc, in_maps, list(range(NCORES)), trace=trace)


def kernel(**inputs) -> np.ndarray:
    in_maps, unshard_info = _host_prep(**inputs)
    res = _run(in_maps)
    out = np.zeros((N, NCLS), dtype=np.float32)
    plane = np.empty((NCLS, COLS), dtype=np.float32)
    for c in range(NCORES):
        nodes_real, cols_real = unshard_info[c]
        y = np.asarray(res.results[c]["y"], dtype=np.float32)  # [128, GRP]
        for g in range(4):
            plane[:, g * GRP:(g + 1) * GRP] = y[32 * g:32 * g + NCLS, :]
        out[nodes_real] = plane[:, cols_real].T
    return out
